# revision 5
# baseline (speedup 1.0000x reference)
"""CausalShapedAttention Trainium2 kernel, v3.

y = beta * softmax(causal(q k^T / sqrt(hd))) @ v + alpha * v - gamma * MC @ v
q,k = x @ W_attn^T (packed), v = x per head, MC = causal uniform attention.

Per core: 2 heads (h0 = 2*core, h0+1), both batches. Output computed
TRANSPOSED (y^T [d, t] per (b, head)); host transposes back (free).

Key structure vs the 180us baseline (which was bound by 969 small matmuls
at the 1.2GHz mid p-state plus 120us of ACT exp):
  - projection in fp8 DoubleRow (K=256 per matmul, half the instructions)
  - scores^T in bf16 K=64 as before, but causal masking is folded into the
    PSUM accumulation via identity-stationary matmuls adding -60 above the
    diagonal (no DVE masking of big tiles)
  - exp on ACT writes fp8e4 P^T pair tiles [128j, 2, W]; U^T = v^T @ P runs
    as fp8 DoubleRow with v as the 65-row stationary and P^T as the 512-wide
    moving operand (baseline did 544 N=65 matmuls, LDW-bound)
  - last jb pairs (g in DVE_G) are exp'd on DVE instead via the Schraudolph
    int16 bit trick: bf16(e^s) bits ~= round(s*184.665 + 16250.4) written as
    int16, tile bitcast to bf16 for plain bf16 U matmuls. Offloads ~18% of
    exp columns from the ACT bottleneck.
  - alpha*v - gamma*MC@v is folded into per-block constant matmuls:
    out^T[:, ib] = v_ib^T @ M'_ib + pfx * cneg_ib (rank-1), with
    M'_ib = alpha*I - gamma*triu/(i+1) host-precomputed in bf16.
  - softmax denominators come from a leading ones-column in v (row 0 of the
    U^T psum); beta/rowsum is broadcast across partitions with a rank-1
    matmul (lhsT = beta-vector, rhs = bf16 reciprocal row).
All emission is hand-interleaved so the PE stream stays dense (p-state!).
"""

import os
import sys
import types

sys.path.insert(0, "/opt/trn_rl_repo")

import numpy as np
import ml_dtypes

B, T, C, H, HD = 2, 2048, 1024, 16, 64
NCORES = 8
HPC = 2                      # heads per core
TB = 16                      # 128-row blocks
NG = 8                       # jb pairs
NW4 = 4                      # 512-wide output blocks
KQ, KK = 256.0, 32.0         # pow2 prescales for fp8 W (q side folds hd^-0.5)
A16 = float(2.0 ** 7 / np.log(2.0))
B16 = 16250.4 + 0.49         # +0.49: emulate round under truncating convert
DVE_G = (6, 7)               # jb pairs exp'd on DVE via int16 bit trick
NEGM = -60.0                 # causal mask addend (exp(-60+3) -> 0 in fp8)

_PROGRAM = None
LAST_EXEC_NS = None
LAST_TRACE_DIR = None


def _wg(g):
    """Stored width of jb pair g (both halves share the even block's width)."""
    return 2048 - 256 * g


def _install_patches():
    try:
        import antenv  # noqa: F401
        if "antenv.axon_hooks" not in sys.modules:
            hooks_mod = types.ModuleType("antenv.axon_hooks")
            _h = [None]
            hooks_mod.set_axon_ntff_profile_hook = lambda h: _h.__setitem__(0, h)
            hooks_mod.get_axon_ntff_profile_hook = lambda: _h[0]
            sys.modules["antenv.axon_hooks"] = hooks_mod
            antenv.axon_hooks = hooks_mod
            from trn_agent_boot.trn_boot import _ntff_profile_via_ctypes
            hooks_mod.set_axon_ntff_profile_hook(
                _ntff_profile_via_ctypes("/opt/axon/libaxon_pjrt.so")
            )
        import concourse.bass_utils as bu
        bu.upload_artifacts = lambda d: d
    except Exception:
        pass


def _split_excess_waits(nc, limit=1):
    """walrus rejects instructions with >1-2 sem waits; split extras onto
    same-engine NoOps just before the instruction."""
    import concourse.mybir as mybir

    n = 0
    for bb in nc.main_func.blocks:
        out = []
        for inst in bb.instructions:
            si = inst.sync_info
            if (
                si is not None
                and si.on_wait
                and len(si.on_wait) > limit
                and inst.engine != mybir.EngineType.Unassigned
            ):
                waits = list(si.on_wait)
                for w in waits[:-limit]:
                    n += 1
                    nop = mybir.InstNoOp(
                        name=f"{inst.name}-wsplit{n}",
                        engine=inst.engine,
                        ins=[], outs=[],
                        sync_info=mybir.SyncInfo(on_wait=[w], on_update=[]),
                    )
                    nc.register_instruction(nop)
                    out.append(nop)
                inst.sync_info = mybir.SyncInfo(
                    on_wait=waits[-limit:], on_update=list(si.on_update)
                )
            out.append(inst)
        bb.instructions = out


def _build_program():
    import concourse.bass as bass
    import concourse.mybir as mybir
    import concourse.tile as tile
    from concourse.bass import ts, ds

    f32 = mybir.dt.float32
    bf16 = mybir.dt.bfloat16
    fp8 = mybir.dt.float8e4
    i16 = mybir.dt.int16
    Exp = mybir.ActivationFunctionType.Exp
    mult = mybir.AluOpType.mult
    add = mybir.AluOpType.add
    DR = mybir.MatmulPerfMode.DoubleRow

    nc = bass.Bass()

    # ---- DRAM ----
    xp8_d = nc.dram_tensor("xp8", [B, 4, 128, 4096], fp8, kind="ExternalInput")
    w8_d = nc.dram_tensor("w8", [128, 2048], fp8, kind="ExternalInput")
    v8_d = nc.dram_tensor("v8", [B, HPC, 128, TB * 80], fp8, kind="ExternalInput")
    v16_d = nc.dram_tensor("v16", [B, HPC, 128, TB * 65], bf16, kind="ExternalInput")
    mprime_d = nc.dram_tensor("mprime", [128, 2048], bf16, kind="ExternalInput")
    cneg_d = nc.dram_tensor("cneg", [1, 2048], bf16, kind="ExternalInput")
    trineg_d = nc.dram_tensor("trineg", [128, 128], bf16, kind="ExternalInput")
    masko_d = nc.dram_tensor("masko", [128, 256], bf16, kind="ExternalInput")
    i128_d = nc.dram_tensor("i128", [128, 128], bf16, kind="ExternalInput")
    ones_d = nc.dram_tensor("ones128", [128, 1], bf16, kind="ExternalInput")
    tri16_d = nc.dram_tensor("tri16", [128, 128], bf16, kind="ExternalInput")
    yT_d = nc.dram_tensor("yT", [B, HPC, 64, T], f32, kind="ExternalOutput")
    rrd_d = nc.dram_tensor("rrd", [4, NW4, 1, 512], bf16, kind="Internal")

    insts = [(0, 0), (0, 1), (1, 0), (1, 1)]  # (b, hs)

    with tile.TileContext(nc) as tc:
        with (
            tc.tile_pool(name="consts", bufs=1) as consts,
            tc.tile_pool(name="xp", bufs=2) as xpp,
            tc.tile_pool(name="vp", bufs=1) as vp,
            tc.tile_pool(name="qk", bufs=1) as qkp,
            tc.tile_pool(name="pt", bufs=2) as ptp,
            tc.tile_pool(name="pfx", bufs=2) as pfxp,
            tc.tile_pool(name="rr", bufs=2) as rrp,
            tc.tile_pool(name="rbs", bufs=4) as rbsp,
            tc.tile_pool(name="ysb", bufs=4) as ysbp,
            tc.tile_pool(name="ps_sc", bufs=2, space="PSUM") as ps_sc,
            tc.tile_pool(name="ps_up", bufs=2, space="PSUM") as ps_up,
            tc.tile_pool(name="ps_mp", bufs=2, space="PSUM") as ps_mp,
        ):
            # ---- Phase A: DMAs ----
            w8_t = consts.tile([128, 2048], fp8, tag="w8", name="w8_t")
            nc.sync.dma_start(w8_t[:], w8_d[:])
            xp_t = {}
            for cp in range(4):
                t = xpp.tile([128, 4096], fp8, tag=f"xp{cp}", name=f"xp0_{cp}")
                nc.sync.dma_start(t[:], xp8_d[0, cp])
                xp_t[0, cp] = t
            v16_t = {}
            v8_t = {}
            for k in insts:
                v16_t[k] = vp.tile([128, TB * 65], bf16, tag=f"v16_{k}",
                                   name=f"v16_{k[0]}_{k[1]}")
                nc.sync.dma_start(v16_t[k][:], v16_d[k[0], k[1]])
            for k in insts:
                v8_t[k] = vp.tile([128, TB * 80], fp8, tag=f"v8_{k}",
                                  name=f"v8_{k[0]}_{k[1]}")
                nc.sync.dma_start(v8_t[k][:], v8_d[k[0], k[1]])
            mprime_t = consts.tile([128, 2048], bf16, tag="mprime", name="mprime_t")
            nc.sync.dma_start(mprime_t[:], mprime_d[:])
            cneg_t = consts.tile([1, 2048], bf16, tag="cneg", name="cneg_t")
            nc.sync.dma_start(cneg_t[:], cneg_d[:])
            trineg_t = consts.tile([128, 128], bf16, tag="trineg", name="trineg_t")
            nc.sync.dma_start(trineg_t[:], trineg_d[:])
            masko_t = consts.tile([128, 256], bf16, tag="masko", name="masko_t")
            nc.sync.dma_start(masko_t[:], masko_d[:])
            i128_t = consts.tile([128, 128], bf16, tag="i128", name="i128_t")
            nc.sync.dma_start(i128_t[:], i128_d[:])
            ones_t = consts.tile([128, 1], bf16, tag="ones", name="ones_t")
            nc.sync.dma_start(ones_t[:], ones_d[:])
            tri16_t = consts.tile([128, 128], bf16, tag="tri16", name="tri16_t")
            nc.sync.dma_start(tri16_t[:], tri16_d[:])
            for cp in range(4):
                t = xpp.tile([128, 4096], fp8, tag=f"xp{cp}", name=f"xp1_{cp}")
                nc.sync.dma_start(t[:], xp8_d[1, cp])
                xp_t[1, cp] = t

            # ---- tiles ----
            qk_t = {}
            for b in range(B):
                for m in range(2):  # 0=q (scaled), 1=k
                    qk_t[b, m] = qkp.tile([128, T], bf16, tag=f"qk{b}{m}",
                                          name=f"qk{b}{m}")
            pt_t = {}  # key (kidx, g) created on the fly (tag per g, bufs=2)
            pfx16 = {}  # (k, ib) -> [1, 65] bf16
            rr_t = {}
            up_ps = {}
            mp_ps = {}

            def w8v(m, cp):
                return w8_t[:, ds((m * 4 + cp) * 256, 256)].rearrange(
                    "p (t c) -> p t c", t=2)

            def v8v(k, g):
                # blocks padded to stride 80 (dual-fp8 LDW needs 16B-aligned
                # even outer step); slice back to the 65 used columns
                return v8_t[k][:, ds(g * 160, 160)].rearrange(
                    "p (t c) -> p t c", t=2)[:, :, ds(0, 65)]

            # ---- emission units ----
            def proj_chain(b, m, n):
                pj = ps_sc.tile([128, 1024], f32, tag="sc", name=f"pj{b}{m}{n}")
                for cp in range(4):
                    nc.tensor.matmul(
                        pj[:, ds(0, 512)], w8v(m, cp),
                        xp_t[b, cp].rearrange("p (t c) -> p t c", t=2)
                        [:, :, ds(n * 512, 512)],
                        start=(cp == 0), stop=(cp == 3), perf_mode=DR,
                    )
                scale = (1.0 / KQ) if m == 0 else (1.0 / KK)
                dst = qk_t[b, m][:, ts(n, 512)]
                if b == 0:
                    if n % 2 == 0:
                        nc.scalar.mul(dst, pj[:, ds(0, 512)], scale)
                    else:
                        nc.vector.tensor_scalar_mul(dst, pj[:, ds(0, 512)], scale)
                else:
                    nc.vector.tensor_scalar_mul(dst, pj[:, ds(0, 512)], scale)

            def cs_unit(k):
                # block colsums + prefix partial sums (all tiny)
                css = []
                for qq in range(4):
                    cp = ps_up.tile([1, 260], f32, tag="up", name=f"cs{k}{qq}")
                    nc.tensor.matmul(cp[:], ones_t[:],
                                     v16_t[k][:, ds(qq * 260, 260)],
                                     start=True, stop=True)
                    sb = pfxp.tile([1, 260], f32, tag="cs_sb", bufs=8,
                                   name=f"cssb{k}{qq}")
                    nc.vector.tensor_copy(sb[:], cp[:])
                    css.append(sb)
                prev = None
                for ib in range(1, TB):
                    s = css[(ib - 1) // 4][0:1, ds(((ib - 1) % 4) * 65, 65)]
                    a = pfxp.tile([1, 65], f32, tag="acc", bufs=2,
                                  name=f"acc{k}{ib}")
                    if prev is None:
                        nc.vector.tensor_copy(a[:], s)
                    else:
                        nc.vector.tensor_add(a[:], prev[:], s)
                    prev = a
                    p16 = pfxp.tile([1, 65], bf16, tag=f"pfx{ib}", bufs=2,
                                    name=f"pfx{k}{ib}")
                    nc.vector.tensor_copy(p16[:], a[:])
                    pfx16[k, ib] = p16

            def s_unit(ki, k, g):
                """scores + exp for jb pair g of instance k."""
                b, hs = k
                wg = _wg(g)
                cs0 = 256 * g
                dve = g in DVE_G
                if (ki, g) not in pt_t:
                    if dve:
                        pt_t[ki, g] = ptp.tile([128, 2 * wg], i16, tag=f"pt{g}",
                                               name=f"pt{ki}_{g}")
                    else:
                        pt_t[ki, g] = ptp.tile([128, 2 * wg], fp8, tag=f"pt{g}",
                                               name=f"pt{ki}_{g}")
                pt = pt_t[ki, g]
                for c0 in range(0, wg, 1024):
                    cw = min(1024, wg - c0)
                    for half in range(2):
                        jb = 2 * g + half
                        sc = ps_sc.tile([128, 1024], f32, tag="sc",
                                        name=f"sc{ki}{g}{half}{c0}")
                        nmm = list(range(0, cw, 512))
                        for o in nmm:
                            no = min(512, cw - o)
                            masked = (c0 == 0 and o == 0 and not dve)
                            nc.tensor.matmul(
                                sc[:, ds(o, no)],
                                qk_t[b, 1][ds(64 * hs, 64), ds(jb * 128, 128)],
                                qk_t[b, 0][ds(64 * hs, 64), ds(cs0 + c0 + o, no)],
                                start=True, stop=not masked,
                                skip_group_check=True,
                            )
                            if masked:
                                mk = trineg_t[:] if half == 0 else masko_t[:]
                                mw = 128 if half == 0 else 256
                                nc.tensor.matmul(
                                    sc[:, ds(0, mw)], i128_t[:], mk,
                                    start=False, stop=True,
                                    skip_group_check=True,
                                )
                        dst = pt[:, ds(half * wg + c0, cw)]
                        if dve:
                            nc.vector.tensor_scalar(
                                dst, sc[:, ds(0, cw)], A16, B16, mult, add)
                            if c0 == 0:
                                # masks are SBUF-only -> gpsimd (PSUM is
                                # off-limits to gpsimd, so it is idle anyway)
                                ptb = pt.bitcast(bf16)
                                if half == 0:
                                    nc.gpsimd.tensor_mul(
                                        ptb[:, ds(0, 128)], ptb[:, ds(0, 128)],
                                        tri16_t[:])
                                else:
                                    nc.gpsimd.memset(pt[:, ds(wg, 128)], 0)
                                    nc.gpsimd.tensor_mul(
                                        ptb[:, ds(wg + 128, 128)],
                                        ptb[:, ds(wg + 128, 128)], tri16_t[:])
                        else:
                            nc.scalar.activation(dst, sc[:, ds(0, cw)], Exp)

            def u_unit(ki, k, iw):
                up = ps_up.tile([65, 512], f32, tag="up", name=f"up{ki}{iw}")
                up_ps[ki, iw] = up
                steps = []
                for g in range(0, 2 * iw + 2):
                    cs0 = 256 * g
                    off = 512 * iw - cs0
                    if off >= 0:
                        steps.append((g, off, 0, 512))
                    else:
                        steps.append((g, 0, 256, 256))
                last_i = len(steps) - 1
                for si, (g, off, dst, n) in enumerate(steps):
                    stop = si == last_i
                    if g not in DVE_G:
                        nc.tensor.matmul(
                            up[:, ds(dst, n)], v8v(k, g),
                            pt_t[ki, g].rearrange("p (t c) -> p t c", t=2)
                            [:, :, ds(off, n)],
                            start=(si == 0), stop=stop, perf_mode=DR,
                            skip_group_check=True,
                        )
                    else:
                        wg = _wg(g)
                        ptb = pt_t[ki, g].bitcast(bf16)
                        for t in range(2):
                            nc.tensor.matmul(
                                up[:, ds(dst, n)],
                                v16_t[k][:, ds((2 * g + t) * 65, 65)],
                                ptb[:, ds(t * wg + off, n)],
                                start=False, stop=(stop and t == 1),
                                skip_group_check=True,
                            )

            def m_unit(ki, k, iw):
                mp = ps_mp.tile([65, 512], f32, tag="mp", name=f"mp{ki}{iw}")
                mp_ps[ki, iw] = mp
                for qq in range(4):
                    ib = 4 * iw + qq
                    nc.tensor.matmul(
                        mp[:, ds(qq * 128, 128)],
                        v16_t[k][:, ds(ib * 65, 65)],
                        mprime_t[:, ds(ib * 128, 128)],
                        start=True, stop=(ib == 0), skip_group_check=True,
                    )
                    if ib > 0:
                        nc.tensor.matmul(
                            mp[:, ds(qq * 128, 128)],
                            pfx16[k, ib][:],
                            cneg_t[0:1, ds(ib * 128, 128)],
                            start=False, stop=True, skip_group_check=True,
                        )

            def recip_unit(ki, k, iw):
                if ki not in rr_t:
                    rr_t[ki] = rrp.tile([1, 2048], bf16, tag="rr", name=f"rr{ki}")
                with nc.allow_low_precision("bf16 recip of softmax denom"):
                    nc.vector.reciprocal(
                        rr_t[ki][0:1, ds(iw * 512, 512)],
                        up_ps[ki, iw][ds(0, 1), :])

            def rb_unit(ki, k, iw):
                # broadcast bf16 recip row across 65 partitions: bounce the
                # row through DRAM, then read back with a 0-stride outer dim
                nc.sync.dma_start(rrd_d[ki, iw],
                                  rr_t[ki][0:1, ds(iw * 512, 512)])
                rbs = rbsp.tile([65, 512], bf16, tag="rbs", name=f"rbs{ki}{iw}")
                nc.sync.dma_start(rbs[:],
                                  rrd_d[ki, iw].broadcast_to((65, 512)))
                return rbs

            def fin_unit(ki, k, iw, rbs):
                b, hs = k
                y = ysbp.tile([65, 512], f32, tag="ysb", name=f"y{ki}{iw}")
                nc.vector.tensor_mul(y[:], up_ps[ki, iw][:], rbs[:])
                nc.vector.tensor_add(y[:], y[:], mp_ps[ki, iw][:])
                nc.sync.dma_start(yT_d[b, hs, :, ts(iw, 512)], y[ds(1, 64), :])

            # ---- orchestration ----
            for m in (1, 0):
                for n in range(4):
                    proj_chain(0, m, n)
            for k in insts:
                cs_unit(k)

            rb_hold = {}
            b1_chains = [(m, n) for m in (1, 0) for n in range(4)]
            for ki in range(len(insts) + 1):
                k = insts[ki] if ki < len(insts) else None
                kp = insts[ki - 1] if ki >= 1 else None
                kpi = ki - 1

                def prev_slot(slot):
                    # PE work for previous instance interleaved between S units
                    if kp is None:
                        return
                    if slot == 0:
                        u_unit(kpi, kp, 0)
                        m_unit(kpi, kp, 0)
                        recip_unit(kpi, kp, 0)
                        rb_hold[kpi, 0] = rb_unit(kpi, kp, 0)
                    elif slot == 1:
                        u_unit(kpi, kp, 1)
                        m_unit(kpi, kp, 1)
                        recip_unit(kpi, kp, 1)
                        rb_hold[kpi, 1] = rb_unit(kpi, kp, 1)
                        fin_unit(kpi, kp, 0, rb_hold[kpi, 0])
                    elif slot == 2:
                        u_unit(kpi, kp, 2)
                        m_unit(kpi, kp, 2)
                        recip_unit(kpi, kp, 2)
                        rb_hold[kpi, 2] = rb_unit(kpi, kp, 2)
                        fin_unit(kpi, kp, 1, rb_hold[kpi, 1])
                    elif slot == 3:
                        u_unit(kpi, kp, 3)
                        m_unit(kpi, kp, 3)
                        recip_unit(kpi, kp, 3)
                        rb_hold[kpi, 3] = rb_unit(kpi, kp, 3)
                        fin_unit(kpi, kp, 2, rb_hold[kpi, 2])
                        fin_unit(kpi, kp, 3, rb_hold[kpi, 3])

                if k is None:
                    for slot in range(4):
                        prev_slot(slot)
                    continue
                for g in range(NG):
                    if ki == 1 and b1_chains:
                        m, n = b1_chains.pop(0)
                        proj_chain(1, m, n)
                    s_unit(ki, k, g)
                    if g == 1:
                        prev_slot(0)
                    elif g == 3:
                        prev_slot(1)
                    elif g == 5:
                        prev_slot(2)
                    elif g == 7:
                        prev_slot(3)
                if ki == 1:
                    while b1_chains:
                        m, n = b1_chains.pop(0)
                        proj_chain(1, m, n)

    _split_excess_waits(nc)
    nc.finalize()
    return nc


def _prep_inputs(x, W_attn, alpha, beta, gamma):
    bf = ml_dtypes.bfloat16
    e4 = ml_dtypes.float8_e4m3
    x = np.asarray(x, dtype=np.float32)
    W_attn = np.asarray(W_attn, dtype=np.float32)
    alpha = float(alpha)
    beta = float(beta)
    gamma = float(gamma)
    sc = HD ** -0.5

    # shared across cores
    # xp8[b, cp, p, t*2048 + n] = x[b, n, 256cp + 128t + p]
    xT = x.transpose(0, 2, 1)  # [B, C, T]
    xp8 = np.ascontiguousarray(
        xT.reshape(B, 4, 2, 128, T).transpose(0, 1, 3, 2, 4).reshape(B, 4, 128, 4096)
    ).astype(e4)

    ar = np.arange(128)
    cinv_full = 1.0 / np.arange(1, T + 1, dtype=np.float32)
    mprime = np.zeros((128, 2048), np.float32)
    for ib in range(TB):
        blk = alpha * np.eye(128, dtype=np.float32) \
            - gamma * np.triu(np.ones((128, 128), np.float32)) \
            * cinv_full[None, ib * 128:(ib + 1) * 128]
        mprime[:, ib * 128:(ib + 1) * 128] = blk
    mprime = mprime.astype(bf)
    cneg = (-gamma * cinv_full).reshape(1, T).astype(bf)
    trineg = np.where(ar[:, None] <= ar[None, :], 0.0, NEGM).astype(np.float32)
    masko = np.concatenate(
        [np.full((128, 128), NEGM, np.float32), trineg], axis=1).astype(bf)
    trineg = trineg.astype(bf)
    i128 = np.eye(128, dtype=np.float32).astype(bf)
    ones128 = np.ones((128, 1), np.float32).astype(bf)
    tri16 = (ar[:, None] <= ar[None, :]).astype(np.float32).astype(bf)

    in_maps = []
    for core in range(NCORES):
        h0 = HPC * core
        # w8[p, (m*4+cp)*256 + t*128 + j] = Wm_scaled[j, 256cp+128t+p]
        wq = W_attn[h0 * 64:(h0 + 2) * 64, :] * (sc * KQ)      # [128, C]
        wk = W_attn[C + h0 * 64:C + (h0 + 2) * 64, :] * KK
        w8 = np.empty((128, 2048), np.float32)
        for m, wm in enumerate((wq, wk)):
            # wm [j=128, c=1024] -> [cp, t, p, j]
            wr = wm.T.reshape(4, 2, 128, 128)
            w8[:, m * 1024:(m + 1) * 1024] = (
                wr.transpose(2, 0, 1, 3).reshape(128, 1024))
        w8 = np.ascontiguousarray(w8).astype(e4)

        # v16: ones-col = 1.0, raw v (colsums / prefix / M').
        # v8: ones-col = 8.0, v * (8*beta) -> recip(row0)*U = beta*attv/rowsum
        # with healthy fp8 dynamic range.
        v = np.empty((B, HPC, TB, 128, 65), np.float32)
        for b in range(B):
            for hs in range(HPC):
                h = h0 + hs
                vb = x[b][:, h * 64:(h + 1) * 64].reshape(TB, 128, 64)
                v[b, hs, :, :, 0] = 1.0
                v[b, hs, :, :, 1:] = vb
        v = np.ascontiguousarray(
            v.transpose(0, 1, 3, 2, 4).reshape(B, HPC, 128, TB * 65))
        v8 = np.zeros((B, HPC, 128, TB, 80), np.float32)
        v8[:, :, :, :, :65] = v.reshape(B, HPC, 128, TB, 65)
        v8[:, :, :, :, 0] = 8.0
        v8[:, :, :, :, 1:65] *= 8.0 * beta
        v8 = v8.reshape(B, HPC, 128, TB * 80)
        in_maps.append({
            "xp8": xp8,
            "w8": w8,
            "v8": v8.astype(e4),
            "v16": v.astype(bf),
            "mprime": mprime,
            "cneg": cneg,
            "trineg": trineg,
            "masko": masko,
            "i128": i128,
            "ones128": ones128,
            "tri16": tri16,
        })
    return in_maps


def kernel(x, W_attn, alpha, beta, gamma):
    global _PROGRAM, LAST_EXEC_NS, LAST_TRACE_DIR
    _install_patches()
    from concourse.bass_utils import run_bass_kernel_spmd

    if _PROGRAM is None:
        _PROGRAM = _build_program()
    nc = _PROGRAM

    in_maps = _prep_inputs(x, W_attn, alpha, beta, gamma)

    trace = os.environ.get("KERNEL_TRACE", "0") == "1"
    kwargs = {}
    if trace:
        trace_dir = os.environ.get("KERNEL_TRACE_DIR") or None
        if trace_dir:
            os.makedirs(trace_dir, exist_ok=True)
            kwargs["tmpdir"] = trace_dir
    res = run_bass_kernel_spmd(
        nc, in_maps, core_ids=list(range(NCORES)), trace=trace, **kwargs
    )
    LAST_EXEC_NS = res.exec_time_ns
    if trace and "tmpdir" in kwargs:
        LAST_TRACE_DIR = kwargs["tmpdir"]

    # yT [B, HPC, 64, T] per core -> y[b, t, core*128 + hs*64 + d]
    out = np.empty((B, T, C), np.float32)
    for core in range(NCORES):
        yT = res.results[core]["yT"]
        out[:, :, core * 128:(core + 1) * 128] = (
            yT.transpose(0, 3, 1, 2).reshape(B, T, 128))
    return np.ascontiguousarray(out)
